# revision 30
# baseline (speedup 1.0000x reference)
"""Trainium2 Bass kernel for CHMSA (cross-covariance multi-head self-attention
with a ConvNorm qkv stem).

Problem (hardcoded):
  x         [16, 64, 64, 256] f32
  dw_kernel [3, 3, 1, 256]    depthwise 3x3, SAME
  bn_gamma/bn_beta [256]      per-channel affine after dwconv
  pw_kernel [256, 768]        1x1 conv -> qkv
  q_bias/v_bias [256]         qkv bias = concat([q_bias, 0, v_bias])
  scale     [8,1,1]           per-head logit scale, s = exp(min(scale, ln 100))
  proj_w    [256, 256], proj_b [256]

Sharding: pure data-parallel over batch: 16 images / 8 cores = 2 images/core.
No collectives.

Per-core dataflow (per image, N = 4096 tokens, C = 256):
  1. DMA x token-major [128,256] tiles; PE-transpose (f32r) -> x^T channel-major.
  2. dwconv: 9 diagonal f32r matmuls per PSUM tile (channel-major), gamma
     folded into the diagonal weights. beta folded into the qkv bias
     constants (exact), so y eviction is a plain cast to bf16 (GpSimd).
  3. qkv: q,k token-major bf16 matmuls (lhsT = y^T columns); v channel-major.
     q/k evicted straight to bf16 (ACT); v evicted with bias (ACT).
  4. l2-normalize: squares on DVE from the bf16 q/k tiles, grouped reduce on
     DVE, single per-token weight w = s_h * rsqrt(max(sq_q*sq_k, eps^2))
     applied to q only.
  5. attn = qs^T k per 4-head group, bf16, free-128 (only the needed half);
     softmax: per-head exp straight from PSUM (ACT) with max subtraction,
     row sums + reciprocal kept for later; 32x32 DVE transposes -> attn^T.
  6. out_cm = attn^T-weighted v (bf16); the softmax 1/rowsum is folded into
     the ocm eviction as a per-partition scale. proj (bf16) back to
     token-major, staged 4 tiles per output DMA.
"""

import math

import numpy as np
import ml_dtypes

import concourse.bass as bass
import concourse.mybir as mybir
import concourse.tile as tile
from concourse import bacc
from concourse.bass_utils import run_bass_kernel_spmd

F32 = mybir.dt.float32
F32R = mybir.dt.float32r
BF16 = mybir.dt.bfloat16
AF = mybir.ActivationFunctionType
ALU = mybir.AluOpType

B, H, W, C = 16, 64, 64, 256
N = H * W              # 4096 tokens per image
HEADS = 8
HD = C // HEADS        # 32
NCORES = 8
IMGS = B // NCORES     # 2 images per core
NCH = C // 128         # 2 channel chunks
LOG_MAX_SCALE = float(np.log(100.0))
L2_EPS = 1e-12

# dwconv tap offsets (dh, dw), center first so it can carry start=True with
# full-tile coverage; the ragged edge taps then accumulate.
TAPS = [(0, 0), (-1, -1), (-1, 0), (-1, 1), (0, -1), (0, 1), (1, -1), (1, 0), (1, 1)]

HBLK = 8               # h-rows per dwconv psum tile -> free dim 8*64 = 512
NBLK = N // 128        # 32 token chunks of 128


def _r(ap):
    """View an fp32 AP as float32r for full-rate PE matmuls."""
    return ap.bitcast(F32R)


def _build_program(consts, add_qkbias, add_pbias, reps=1):
    nc = bacc.Bacc()

    x_dr = nc.dram_tensor("x", [IMGS, N, C], F32, kind="ExternalInput")
    out_dr = nc.dram_tensor("out", [IMGS, N, C], F32, kind="ExternalOutput")

    diag_dr = nc.inline_tensor(consts["diag"], "cdiag")        # [128, NCH, 9, 128] f32
    pwqk_dr = nc.inline_tensor(consts["pwqk"], "cpwqk")        # [128, NCH, 512] bf16
    pwv_dr = nc.inline_tensor(consts["pwv"], "cpwv")           # [128, NCH, NCH, 128] bf16
    projw_dr = nc.inline_tensor(consts["projw"], "cprojw")     # [128, NCH, 256] bf16
    vb_dr = nc.inline_tensor(consts["vb"], "cvb")              # [128, NCH] f32
    ident_dr = nc.inline_tensor(consts["ident"], "cident")     # [128, 128] f32
    srep_dr = nc.inline_tensor(consts["srep"], "csrep")        # [128, 32*8] f32
    if add_qkbias:
        qkb_dr = nc.inline_tensor(consts["qkb"], "cqkb")       # [128, 512] f32
    if add_pbias:
        pb_dr = nc.inline_tensor(consts["pb"], "cpb")          # [128, 256] f32

    with tile.TileContext(nc) as tc:
        with (
            tc.tile_pool(name="singles", bufs=1) as singles,
            tc.tile_pool(name="xstage", bufs=3) as xstage,
            tc.tile_pool(name="xt", bufs=1) as xt_pool,
            tc.tile_pool(name="img_big", bufs=2) as img_pool,
            tc.tile_pool(name="qkt", bufs=3) as qkt_pool,
            tc.tile_pool(name="qs", bufs=2) as qs_pool,
            tc.tile_pool(name="sqs", bufs=3) as sqs_pool,
            tc.tile_pool(name="small", bufs=3) as small,
            tc.tile_pool(name="atp", bufs=2) as at_pool,
            tc.tile_pool(name="ocm", bufs=3) as ocm_pool,
            tc.tile_pool(name="ostage", bufs=2) as ostage,
            tc.tile_pool(name="ps_mm", bufs=4, space="PSUM") as ps_mm,
            tc.tile_pool(name="ps_c", bufs=2, space="PSUM") as ps_c,
            tc.tile_pool(name="ps_attn", bufs=2, space="PSUM") as ps_attn,
        ):
            # ---- constants into SBUF ----
            ident_sb = singles.tile([128, 128], F32R)
            nc.gpsimd.dma_start(ident_sb[:], _r(ident_dr[:]))
            diag_sb = singles.tile([128, NCH, 9, 128], F32)
            nc.gpsimd.dma_start(diag_sb[:], diag_dr[:])
            pwqk_sb = singles.tile([128, NCH, 512], F32)
            nc.gpsimd.dma_start(pwqk_sb[:], pwqk_dr[:])
            pwv_sb = singles.tile([128, NCH, NCH, 128], F32)
            nc.gpsimd.dma_start(pwv_sb[:], pwv_dr[:])
            projw_sb = singles.tile([128, NCH, 256], BF16)
            nc.gpsimd.dma_start(projw_sb[:], projw_dr[:])
            vb_sb = singles.tile([128, NCH], F32)
            nc.gpsimd.dma_start(vb_sb[:], vb_dr[:])
            srep_sb = singles.tile([128, 32 * 8], F32)
            nc.gpsimd.dma_start(srep_sb[:], srep_dr[:])
            if add_qkbias:
                qkb_sb = singles.tile([128, 512], F32)
                nc.gpsimd.dma_start(qkb_sb[:], qkb_dr[:])
            if add_pbias:
                pb_sb = singles.tile([128, 256], F32)
                nc.gpsimd.dma_start(pb_sb[:], pb_dr[:])

            # warm the ACT function tables up front so the 1.3us table loads
            # don't land mid-stream
            warm = singles.tile([128, 2], F32)
            nc.vector.memset(warm[:], 0.0)
            nc.scalar.activation(warm[:, 0:1], warm[:, 0:1], AF.Identity)
            nc.scalar.activation(warm[:, 0:1], warm[:, 0:1], AF.Exp)
            nc.scalar.activation(warm[:, 0:1], warm[:, 0:1], AF.Sqrt)

            def make_img_state(img):
                st = {}
                st["img"] = img
                st["xt"] = xt_pool.tile([128, NCH, H + 2, W + 2], F32, tag="xt",
                                        name=f"xt_{img}")
                nc.vector.memset(st["xt"][:, :, 0, :], 0.0)
                nc.vector.memset(st["xt"][:, :, H + 1, :], 0.0)
                nc.vector.memset(st["xt"][:, :, :, 0], 0.0)
                nc.vector.memset(st["xt"][:, :, :, W + 1], 0.0)
                st["yt"] = img_pool.tile([128, NCH, N], F32, tag="yt",
                                         name=f"yt{img}")
                st["vt"] = img_pool.tile([128, NCH, N], BF16, tag="vt",
                                         name=f"vt{img}")
                st["sq"] = img_pool.tile([128, NBLK, 16], F32, tag="sqall",
                                         name=f"sq{img}")
                st["w"] = img_pool.tile([128, NBLK, 8], F32, tag="wall",
                                        name=f"w{img}")
                st["att"] = [ps_attn.tile([128, 128], F32, tag="att",
                                          name=f"att{img}_{g}")
                             for g in range(2)]
                st["at_bd"] = at_pool.tile([128, 2, 128], BF16, tag="atbd",
                                           name=f"atbd{img}")
                st["rsum"] = at_pool.tile([128, 2, 1], F32, tag="rsum",
                                          name=f"rsum{img}")
                st["qkt"] = {}
                return st

            def load_transpose(st, tb):
                img = st["img"]
                stg = xstage.tile([128, 2, C], F32R, name="stg")
                nc.sync.dma_start(
                    stg[:],
                    _r(x_dr[img, tb * 256:(tb + 1) * 256, :]).rearrange(
                        "(g p) c -> p g c", p=128),
                )
                tp = ps_mm.tile([128, 512], F32, tag="mm", name="tp")
                for g in range(2):
                    for cch in range(NCH):
                        nc.tensor.transpose(
                            _r(tp[:, cch * 256 + g * 128:cch * 256 + g * 128 + 128]),
                            stg[:, g, cch * 128:cch * 128 + 128],
                            ident_sb[:],
                        )
                r0 = tb * 4   # 256 tokens = 4 h-rows
                nc.scalar.copy(
                    _r(st["xt"][:, :, 1 + r0:1 + r0 + 4, 1:1 + W]),
                    tp[:],
                )

            def dwconv_block(st, hb):
                h0 = hb * HBLK
                for cch in range(NCH):
                    yp = ps_mm.tile([128, HBLK * W], F32, tag="mm", name="yp")
                    for ti, (dh, dw) in enumerate(TAPS):
                        nc.tensor.matmul(
                            yp[:],
                            _r(diag_sb[:, cch, ti, :]),
                            _r(st["xt"][:, cch, 1 + h0 + dh:1 + h0 + HBLK + dh,
                                         1 + dw:1 + W + dw]),
                            start=(ti == 0),
                            stop=(ti == len(TAPS) - 1),
                            skip_group_check=True,
                        )
                    # beta folded into qkv biases: plain copy eviction
                    nc.vector.tensor_copy(
                        _r(st["yt"][:, cch, h0 * W:(h0 + HBLK) * W]), yp[:])

            def v_block(st, nb):
                for vc in range(NCH):
                    vp = ps_mm.tile([128, 512], F32, tag="mm", name="vp")
                    for kc in range(NCH):
                        nc.tensor.matmul(
                            vp[:],
                            _r(pwv_sb[:, kc, vc, :]),
                            _r(st["yt"][:, kc, nb * 512:(nb + 1) * 512]),
                            start=(kc == 0),
                            stop=(kc == NCH - 1),
                        )
                    nc.scalar.activation(
                        out=st["vt"][:, vc, nb * 512:(nb + 1) * 512],
                        in_=vp[:],
                        func=AF.Identity,
                        bias=vb_sb[:, vc:vc + 1],
                    )

            def qk_block(st, hb, t):
                qkt = st["qkt"][hb]
                qp = ps_mm.tile([128, 512], F32, tag="mm", name="qp")
                for kc in range(NCH):
                    nc.tensor.matmul(
                        qp[:],
                        _r(st["yt"][:, kc, t * 128:(t + 1) * 128]),
                        _r(pwqk_sb[:, kc, :]),
                        start=(kc == 0),
                        stop=(kc == NCH - 1),
                    )
                if add_qkbias:
                    nc.vector.tensor_tensor(
                        out=qkt[:, t % 4, :], in0=qp[:], in1=qkb_sb[:],
                        op=ALU.add)
                else:
                    nc.scalar.activation(qkt[:, t % 4, :], qp[:], AF.Identity)

            def sq_batch(st, hb):
                # squares + grouped reduce for the whole hb (4 token tiles)
                qkt = st["qkt"][hb]
                sqs = sqs_pool.tile([128, 4, 512], BF16, name="sqs")
                nc.vector.tensor_tensor(
                    out=sqs[:], in0=qkt[:], in1=qkt[:], op=ALU.mult)
                nc.vector.tensor_reduce(
                    out=st["sq"][:, 4 * hb:4 * hb + 4, :],
                    in_=sqs.rearrange("p t (g d) -> p t g d", d=HD),
                    axis=mybir.AxisListType.X,
                    op=ALU.add,
                )

            def w_batch(st, b):
                # normalization weight for chunks [4b, 4b+4)
                sl = slice(4 * b, 4 * b + 4)
                w_all, sq_all = st["w"], st["sq"]
                nc.vector.tensor_tensor(
                    out=w_all[:, sl, :],
                    in0=sq_all[:, sl, 0:8],
                    in1=sq_all[:, sl, 8:16],
                    op=ALU.mult,
                )
                wf = w_all[:, sl, :].rearrange("p t h -> p (t h)")
                nc.vector.tensor_scalar(
                    out=wf, in0=wf, scalar1=float(L2_EPS * L2_EPS),
                    scalar2=None, op0=ALU.max,
                )
                nc.scalar.activation(wf, wf, AF.Sqrt)
                nc.vector.reciprocal(wf, wf)
                nc.vector.tensor_tensor(
                    out=wf, in0=wf, in1=srep_sb[:, 8 * 4 * b:8 * 4 * (b + 1)],
                    op=ALU.mult)

            def qs_attn_batch(st, hb):
                qkt = st["qkt"][hb]
                qs = qs_pool.tile([128, 4, 256], BF16, tag="qs",
                                  name=f"qs{st['img']}_{hb}")
                for i, t in enumerate(range(4 * hb, 4 * hb + 4)):
                    nc.vector.tensor_tensor(
                        out=qs[:, i, :].rearrange("p (h d) -> p h d", d=HD),
                        in0=qkt[:, i, 0:256].rearrange("p (h d) -> p h d",
                                                       d=HD),
                        in1=st["w"][:, t, :].unsqueeze(2).broadcast_to(
                            [128, 8, HD]),
                        op=ALU.mult,
                    )
                    for g in range(2):
                        nc.tensor.matmul(
                            st["att"][g][:],
                            qs[:, i, g * 128:(g + 1) * 128],
                            qkt[:, i, 256 + g * 128:256 + (g + 1) * 128],
                            start=(t == 0),
                            stop=(t == NBLK - 1),
                            skip_group_check=True,
                        )

            def softmax_at(st):
                at_bd, rsum = st["at_bd"], st["rsum"]
                for g in range(2):
                    asm = small.tile([128, 32], F32, tag="asm", name="asm")
                    for j in range(4):
                        nc.vector.tensor_copy(
                            asm[32 * j:32 * j + 32, :],
                            st["att"][g][32 * j:32 * j + 32,
                                         32 * j:32 * j + 32],
                        )
                    mx = small.tile([128, 1], F32, tag="mx", name="mx")
                    nc.vector.tensor_reduce(
                        out=mx[:], in_=asm[:], axis=mybir.AxisListType.X,
                        op=ALU.max, negate=True)
                    nc.scalar.activation(asm[:], asm[:], AF.Exp, bias=mx[:])
                    sm = small.tile([128, 1], F32, tag="sm", name="sm")
                    nc.vector.tensor_reduce(
                        out=sm[:], in_=asm[:], axis=mybir.AxisListType.X,
                        op=ALU.add)
                    nc.vector.reciprocal(rsum[:, g, :], sm[:])
                    atf = small.tile([128, 128], F32, tag="atf", name="atf")
                    nc.vector.memset(atf[:], 0.0)
                    for j in range(4):
                        nc.vector.transpose(
                            atf[32 * j:32 * j + 32, 32 * j:32 * j + 32],
                            asm[32 * j:32 * j + 32, :],
                        )
                    nc.vector.tensor_copy(at_bd[:, g, :], atf[:])

            def c_block(st, nb, tail=False):
                # one 512-token slab: attn^T @ v (1/rowsum folded in), proj,
                # stage 4 token-tiles, DMA out.
                img = st["img"]
                ocm = ocm_pool.tile([128, NCH, 512], BF16, tag="ocm",
                                    name=f"ocm{img}_{nb}")
                for g in range(NCH):
                    op_ = ps_c.tile([128, 512], F32, tag="cmm", name="op_")
                    nc.tensor.matmul(
                        op_[:],
                        st["at_bd"][:, g, :],
                        st["vt"][:, g, nb * 512:(nb + 1) * 512],
                    )
                    if g == 0:
                        nc.scalar.activation(
                            ocm[:, g, :], op_[:], AF.Copy,
                            scale=st["rsum"][:, g, :])
                    else:
                        nc.vector.tensor_scalar(
                            out=ocm[:, g, :], in0=op_[:],
                            scalar1=st["rsum"][:, g, :], scalar2=None,
                            op0=ALU.mult)
                ot = ostage.tile([128, 4, 256], F32, name="ot")
                for i, t in enumerate(range(4 * nb, 4 * nb + 4)):
                    pp = ps_c.tile([128, 512], F32, tag="cmm", name="pp")
                    ppv = pp[:, 0:256]
                    for kc in range(NCH):
                        nc.tensor.matmul(
                            ppv,
                            ocm[:, kc, i * 128:(i + 1) * 128],
                            projw_sb[:, kc, :],
                            start=(kc == 0),
                            stop=(kc == NCH - 1),
                        )
                    if add_pbias:
                        nc.vector.tensor_tensor(out=ot[:, i, :], in0=ppv,
                                                in1=pb_sb[:], op=ALU.add)
                    elif i % 2:
                        nc.vector.tensor_copy(ot[:, i, :], ppv)
                    else:
                        nc.scalar.activation(ot[:, i, :], ppv, AF.Identity)
                nc.sync.dma_start(
                    out_dr[img, 4 * nb * 128:(4 * nb + 4) * 128, :].rearrange(
                        "(g p) c -> p g c", p=128),
                    ot[:],
                )

            def process_hb(st, hb):
                # attn for the previous block first: its inputs are long
                # ready, so neither PE nor DVE stalls at the queue head
                if hb >= 1:
                    qs_attn_batch(st, hb - 1)
                dwconv_block(st, hb)
                if hb >= 1:
                    v_block(st, hb - 1)
                st["qkt"][hb] = qkt_pool.tile([128, 4, 512], BF16, tag="qkt",
                                              name=f"qkt{st['img']}_{hb}")
                for t in range(4 * hb, 4 * hb + 4):
                    qk_block(st, hb, t)
                sq_batch(st, hb)
                w_batch(st, hb)
                if hb >= 1:
                    del st["qkt"][hb - 1]

            def phase_A(st, interleave=None):
                for tb in range(16):
                    load_transpose(st, tb)
                    if interleave is not None:
                        interleave(tb)
                    if tb >= 4 and tb % 2 == 0:
                        process_hb(st, (tb - 4) // 2)
                for hb in (6, 7):
                    process_hb(st, hb)
                qs_attn_batch(st, 7)
                v_block(st, 7)
                softmax_at(st)

            prev = None
            for img in range(IMGS):
                st = make_img_state(img)
                if prev is None:
                    phase_A(st)
                else:
                    pv = prev

                    def emit_c(tb, pv=pv):
                        if tb < 8:
                            c_block(pv, tb)
                    phase_A(st, interleave=emit_c)
                prev = st
            for nb in range(8):
                c_block(prev, nb, tail=True)

    nc.finalize()
    return nc


def _prep_consts(dw_kernel, bn_gamma, bn_beta, pw_kernel, q_bias, v_bias,
                 scale, proj_w, proj_b):
    taps_w = np.empty((9, C), np.float32)
    for ti, (dh, dw) in enumerate(TAPS):
        taps_w[ti] = dw_kernel[dh + 1, dw + 1, 0, :] * bn_gamma

    diag = np.zeros((128, NCH, 9, 128), np.float32)
    idx = np.arange(128)
    for cch in range(NCH):
        for ti in range(9):
            diag[idx, cch, ti, idx] = taps_w[ti, cch * 128 + idx]

    # beta folded into the qkv bias (exact): qkv = y0 @ pw + (beta @ pw + b)
    qkv_bias = np.concatenate(
        [q_bias, np.zeros_like(q_bias), v_bias]).astype(np.float64)
    qkv_bias = (qkv_bias + bn_beta.astype(np.float64)
                @ pw_kernel.astype(np.float64)).astype(np.float32)

    pwqk = np.empty((128, NCH, 512), np.float32)
    pwv = np.empty((128, NCH, NCH, 128), np.float32)
    for kc in range(NCH):
        pwqk[:, kc, :] = pw_kernel[kc * 128:(kc + 1) * 128, 0:512]
        for vc in range(NCH):
            pwv[:, kc, vc, :] = pw_kernel[
                kc * 128:(kc + 1) * 128,
                512 + vc * 128:512 + (vc + 1) * 128]

    projw = np.empty((128, NCH, 256), ml_dtypes.bfloat16)
    for kc in range(NCH):
        projw[:, kc, :] = proj_w[kc * 128:(kc + 1) * 128, :].astype(
            ml_dtypes.bfloat16)

    s = np.exp(np.minimum(scale.reshape(HEADS), LOG_MAX_SCALE)).astype(np.float32)
    srep = np.tile(np.tile(s, 32)[None, :], (128, 1)).astype(np.float32)
    smax = float(np.max(s))

    consts = {
        "diag": diag,
        "pwqk": pwqk,
        "pwv": pwv,
        "projw": projw,
        "ident": np.eye(128, dtype=np.float32),
        "vb": qkv_bias[512:768].reshape(NCH, 128).T.astype(np.float32).copy(),
        "srep": srep,
        "smax": smax,
        "qkb": np.tile(qkv_bias[None, 0:512], (128, 1)).astype(np.float32),
        "pb": np.tile(proj_b[None, :], (128, 1)).astype(np.float32),
    }
    return consts


def kernel(x, dw_kernel, bn_gamma, bn_beta, pw_kernel, q_bias, v_bias, scale,
           proj_w, proj_b):
    x = np.ascontiguousarray(np.asarray(x, np.float32))
    consts = _prep_consts(
        np.asarray(dw_kernel, np.float32), np.asarray(bn_gamma, np.float32),
        np.asarray(bn_beta, np.float32), np.asarray(pw_kernel, np.float32),
        np.asarray(q_bias, np.float32), np.asarray(v_bias, np.float32),
        np.asarray(scale, np.float32), np.asarray(proj_w, np.float32),
        np.asarray(proj_b, np.float32))

    add_qkbias = bool(np.any(consts["qkb"]))
    add_pbias = bool(np.any(consts["pb"]))
    nc = _build_program(consts, add_qkbias, add_pbias)

    xs = x.reshape(NCORES, IMGS, N, C)
    in_maps = [{"x": np.ascontiguousarray(xs[i])} for i in range(NCORES)]
    res = run_bass_kernel_spmd(nc, in_maps, core_ids=list(range(NCORES)))
    out = np.stack([res.results[i]["out"] for i in range(NCORES)])
    return out.reshape(B, H, W, C)


if __name__ == "__main__":
    pass
